# revision 4
# baseline (speedup 1.0000x reference)
"""Trainium2 Bass kernel v2 for 2-layer edge-featured GAT + mean-pool + FC.

vs v1:
- Per-edge gathers via indirect_dma_start (HW DGE; ~0.2us Pool engine per
  128-row call) instead of DMAGatherAnt Q7 ucode (~38us per 4096 rows).
- Phase 0 replicated: every core computes the FULL node table with one GEMM
  against W1ext (h cols interleaved with ones cols + a_src + a_dst cols), so
  AllGather #1 and the DVE a_src/a_dst reduce chains disappear.
- Table row (bf16): [h0(64)|1|h1(64)|1|h2(64)|1|h3(64)|1|asrc(4)|adst(4)|pad]
  so one broadcast multiply by p yields messages + denominator ride-along.
- No nc.vector.tensor_scalar anywhere (pathologically slow on DVE); relu /
  strided extracts / casts run on the Scalar (Activation) engine.
- One-hots (scatter + pooling) and w*q edge constants host-precomputed.
- Layer-1 a_dst via one-hot-transpose matmuls (PE); layer-2 a_dst via a tiny
  per-chunk indirect gather from the table itself.
"""

import sys

sys.path.insert(0, "/opt/trn_rl_repo")

import math
from contextlib import ExitStack

import numpy as np
import ml_dtypes

import concourse.bacc as bacc
import concourse.bass as bass
import concourse.mybir as mybir
import concourse.tile as tile
from concourse.bass_utils import run_bass_kernel_spmd
from concourse.masks import make_identity

P = 128
NCORES = 8

FULL_CFG = dict(N=20000, E=640000, FIN=128, HID=64, HEADS=4, NG=256, OUT=32)

F32 = mybir.dt.float32
BF16 = mybir.dt.bfloat16
I32 = mybir.dt.int32
BF = ml_dtypes.bfloat16

ROW1 = 384   # bf16 row elems: 4*(64+1)=260 | asrc 4 | adst 4 | pad (768B rows)
ROW2 = 128   # bf16 row elems: 64 | 1 | asrc2 | adst2 | pad (256B rows)
C1 = 268     # live GEMM cols layer 1
C2 = 67      # live GEMM cols layer 2
D1I = 260    # interleaved msg width


# ---------------------------------------------------------------------------
# Host-side preparation (indices + constants only).
# ---------------------------------------------------------------------------
def prepare(inputs, cfg):
    N, E, FIN, HID, HEADS, NG, OUT = (
        cfg["N"], cfg["E"], cfg["FIN"], cfg["HID"], cfg["HEADS"], cfg["NG"],
        cfg["OUT"],
    )
    GPC = NG // NCORES

    x = np.asarray(inputs["x"], np.float32)
    ei = np.asarray(inputs["edge_index"], np.int64)
    ea = np.asarray(inputs["edge_attr"], np.float32)
    batch = np.asarray(inputs["batch"], np.int64)
    src, dst = ei[0], ei[1]

    bounds = np.searchsorted(batch, np.arange(NCORES + 1) * GPC)
    node_cnt = np.diff(bounds)
    NT = max(1, math.ceil(node_cnt.max() / P))
    NSLICE = NT * P
    NROWS = NCORES * NSLICE

    core_of_node = np.minimum(batch // GPC, NCORES - 1).astype(np.int64)
    deg = np.bincount(dst, minlength=N).astype(np.int64)

    # degree-balanced node->tile packing per core (LPT greedy)
    slot_of = np.empty(N, np.int64)
    tile_edges = np.zeros((NCORES, NT), np.int64)
    for c in range(NCORES):
        ns, ne = bounds[c], bounds[c + 1]
        nodes = np.arange(ns, ne)
        order = np.argsort(-deg[nodes], kind="stable")
        loads = np.zeros(NT, np.int64)
        counts = np.zeros(NT, np.int64)
        pos_of = np.empty(len(nodes), np.int64)
        tile_of = np.empty(len(nodes), np.int64)
        for i in order:
            cand = np.where(counts < P)[0]
            t = cand[np.argmin(loads[cand])]
            tile_of[i] = t
            pos_of[i] = counts[t]
            counts[t] += 1
            loads[t] += deg[nodes[i]]
        slot_of[nodes] = tile_of * P + pos_of
        tile_edges[c] = loads

    CPT = max(1, int(math.ceil(tile_edges.max() / P)))
    NCHUNK = NT * CPT
    NI = CPT * P

    rowid = core_of_node * NSLICE + slot_of

    # full-table node features, slot-major, transposed (same for all cores)
    xsT = np.zeros((FIN, NROWS), np.float32)
    xsT[:, rowid] = x.T
    xsT = xsT.astype(BF)

    # edges grouped by (dst core, dst tile)
    ecore = core_of_node[dst]
    etile = slot_of[dst] // P
    ekey = ecore * NT + etile
    order = np.argsort(ekey, kind="stable")
    dsts = dst[order]
    srcs = src[order]
    ws = ea[order, 0]
    ekey_s = ekey[order]
    cell_bounds = np.searchsorted(ekey_s, np.arange(NCORES * NT + 1))

    D1 = HEADS * HID

    # ---- weights ----
    W1 = np.asarray(inputs["W1"], np.float32)
    W2 = np.asarray(inputs["W2"], np.float32)
    as1 = np.asarray(inputs["att_src1"], np.float32)
    ad1 = np.asarray(inputs["att_dst1"], np.float32)
    as2 = np.asarray(inputs["att_src2"], np.float32).reshape(-1)
    ad2 = np.asarray(inputs["att_dst2"], np.float32).reshape(-1)
    q1 = (np.asarray(inputs["We1"], np.float32).reshape(HEADS, HID)
          * np.asarray(inputs["att_edge1"], np.float32)).sum(axis=1)
    q2 = float((np.asarray(inputs["We2"], np.float32).reshape(-1)
                * np.asarray(inputs["att_edge2"], np.float32).reshape(-1))
               .sum())
    b1 = np.asarray(inputs["b1"], np.float32)
    b2 = np.asarray(inputs["b2"], np.float32)
    fcW = np.asarray(inputs["fcW"], np.float32)
    fcb = np.asarray(inputs["fcb"], np.float32)

    W1e = np.zeros((FIN, C1), np.float32)
    for h in range(HEADS):
        W1e[:, 65 * h:65 * h + 64] = W1[:, 64 * h:64 * h + 64]
        W1e[:, 260 + h] = W1[:, 64 * h:64 * h + 64] @ as1[h]
        W1e[:, 264 + h] = W1[:, 64 * h:64 * h + 64] @ ad1[h]
    ones1 = np.zeros((1, C1), np.float32)
    for h in range(HEADS):
        ones1[0, 65 * h + 64] = 1.0

    # W2ext rows follow out1's interleaved layout; denominator-slot rows and
    # pad rows (260..383) are zero.
    W2e_pad = np.zeros((3 * P, C2), np.float32)
    for h in range(HEADS):
        W2e_pad[65 * h:65 * h + 64, 0:HID] = W2[64 * h:64 * h + 64]
        W2e_pad[65 * h:65 * h + 64, 65] = W2[64 * h:64 * h + 64] @ as2
        W2e_pad[65 * h:65 * h + 64, 66] = W2[64 * h:64 * h + 64] @ ad2
    ones2 = np.zeros((1, C2), np.float32)
    ones2[0, 64] = 1.0

    per_core = []
    for c in range(NCORES):
        isrc = np.zeros((NT, NI), np.int64)
        idst = np.zeros((NT, NI), np.int64)
        dstl = np.full((NT, NI), -1.0, np.float32)
        wv = np.zeros((NT, NI), np.float32)
        for t in range(NT):
            es, ee = cell_bounds[c * NT + t], cell_bounds[c * NT + t + 1]
            k = ee - es
            assert k <= NI
            isrc[t, :k] = rowid[srcs[es:ee]]
            idst[t, :k] = rowid[dsts[es:ee]]
            dstl[t, :k] = (slot_of[dsts[es:ee]] % P).astype(np.float32)
            wv[t, :k] = ws[es:ee]

        def perm(a, dtype):
            return np.ascontiguousarray(a.reshape(NCHUNK, P).T).astype(dtype)

        def wrap_idx(arr):  # [NT, NI] -> [128, NT*CPT*8] int16 for dma_gather
            blocks = []
            for t in range(NT):
                a = arr[t].reshape(CPT * 8, 16).T  # [16, CPT*8]
                blocks.append(np.tile(a, (8, 1)))
            return np.ascontiguousarray(
                np.concatenate(blocks, axis=1)).astype(np.int16)

        ixs16 = wrap_idx(isrc)
        dstl_pc = perm(dstl, np.float32)
        wv_pc = perm(wv, np.float32)

        ohd = (dstl_pc[:, :, None] ==
               np.arange(P, dtype=np.float32)[None, None, :])
        ohTd = np.ascontiguousarray(
            ohd.transpose(2, 1, 0).reshape(P, NCHUNK * P)).astype(BF)
        ohd = np.ascontiguousarray(ohd.reshape(P, NCHUNK * P)).astype(BF)

        wvq1 = np.ascontiguousarray(
            (wv_pc[:, :, None] * q1[None, None, :])
            .reshape(P, NCHUNK * HEADS)).astype(np.float32)
        wvq2 = (wv_pc * q2).astype(np.float32)

        ns, ne = bounds[c], bounds[c + 1]
        nodes = np.arange(ns, ne)
        gl = np.full((NSLICE,), -1, np.int64)
        gl[slot_of[nodes]] = batch[nodes] - c * GPC
        ohg = (gl.reshape(NT, P)[:, :, None] ==
               np.arange(GPC)[None, None, :])
        ohg = np.ascontiguousarray(
            ohg.transpose(1, 0, 2).reshape(P, NT * GPC)).astype(BF)

        iloc = (c * NSLICE + np.arange(NT)[None, :] * P
                + np.arange(P)[:, None]).astype(np.int32)  # [P, NT]

        per_core.append(dict(
            ixs=ixs16, ohd=ohd, ohTd=ohTd, wvq1=wvq1, wvq2=wvq2,
            ohg=ohg, iloc=iloc,
        ))

    rep = lambda vv: np.tile(vv[None, :].astype(np.float32), (P, 1)).copy()
    # b1 in interleaved layout (260 wide, denominator slots get 0)
    b1i = np.zeros((D1I,), np.float32)
    for h in range(HEADS):
        b1i[65 * h:65 * h + 64] = b1[64 * h:64 * h + 64]
    consts = dict(
        xsT=xsT,
        W1e=W1e.astype(BF),
        W2e=np.ascontiguousarray(
            W2e_pad.reshape(3, P, C2).transpose(1, 0, 2)).astype(BF),
        ones1=ones1.astype(BF), ones2=ones2.astype(BF),
        b1b=rep(b1i), b2b=rep(b2),
        fcw=fcW, fcbb=rep(fcb),
    )
    in_maps = []
    for c in range(NCORES):
        m = dict(per_core[c])
        m.update(consts)
        in_maps.append(m)

    meta = dict(NT=NT, CPT=CPT, NSLICE=NSLICE, NROWS=NROWS, GPC=GPC, **cfg)
    return in_maps, meta


# ---------------------------------------------------------------------------
# Device program.
# ---------------------------------------------------------------------------
def build(meta, num_devices=NCORES):
    NT, CPT = meta["NT"], meta["CPT"]
    NSLICE, NROWS, GPC = meta["NSLICE"], meta["NROWS"], meta["GPC"]
    FIN, HID, HEADS, OUT = meta["FIN"], meta["HID"], meta["HEADS"], meta["OUT"]
    NTF = NROWS // P
    NCHUNK = NT * CPT
    A = mybir.AluOpType
    ACT = mybir.ActivationFunctionType
    rg = [list(range(NCORES))]
    TB = 2
    NTB = (CPT + TB - 1) // TB
    IOA = bass.IndirectOffsetOnAxis

    nc = bacc.Bacc("TRN2", target_bir_lowering=False, debug=False,
                   num_devices=num_devices,
                   dynamic_dma_scratch_size=65536)

    def din(name, shape, dtype=F32):
        return nc.dram_tensor(name, list(shape), dtype,
                              kind="ExternalInput").ap()

    xsT_d = din("xsT", (FIN, NROWS), BF16)
    ixs_d = din("ixs", (P, NCHUNK * 8), mybir.dt.int16)
    iloc_d = din("iloc", (P, NT), I32)
    ohd_d = din("ohd", (P, NCHUNK * P), BF16)
    ohTd_d = din("ohTd", (P, NCHUNK * P), BF16)
    wvq1_d = din("wvq1", (P, NCHUNK * HEADS))
    wvq2_d = din("wvq2", (P, NCHUNK))
    ohg_d = din("ohg", (P, NT * GPC), BF16)
    W1e_d = din("W1e", (FIN, C1), BF16)
    W2e_d = din("W2e", (P, 3, C2), BF16)
    ones1_d = din("ones1", (1, C1), BF16)
    ones2_d = din("ones2", (1, C2), BF16)
    b1_d = din("b1b", (P, D1I))
    b2_d = din("b2b", (P, HID))
    fcw_d = din("fcw", (HID, OUT))
    fcb_d = din("fcbb", (P, OUT))

    out_d = nc.dram_tensor("out", [GPC, OUT], F32, kind="ExternalOutput").ap()

    with tile.TileContext(nc) as tc, ExitStack() as st:
        constp = st.enter_context(tc.tile_pool(name="constp", bufs=1))
        drp = st.enter_context(tc.tile_pool(name="drp", bufs=1, space="DRAM"))

        identb = constp.tile([P, P], BF16)
        make_identity(nc, identb[:])
        identf = constp.tile([P, P], F32)
        make_identity(nc, identf[:])
        ixs_sb = constp.tile([P, NCHUNK * 8], mybir.dt.int16)
        nc.sync.dma_start(ixs_sb[:], ixs_d[:])
        iloc_sb = constp.tile([P, NT], I32)
        nc.sync.dma_start(iloc_sb[:], iloc_d[:])
        wvq1_sb = constp.tile([P, NCHUNK, HEADS], F32)
        nc.sync.dma_start(
            wvq1_sb[:], wvq1_d[:].rearrange("p (c h) -> p c h", h=HEADS))
        wvq2_sb = constp.tile([P, NCHUNK], F32)
        nc.sync.dma_start(wvq2_sb[:], wvq2_d[:])
        ohg_sb = constp.tile([P, NT, GPC], BF16)
        nc.sync.dma_start(ohg_sb[:],
                          ohg_d[:].rearrange("p (t g) -> p t g", g=GPC))
        onesc = constp.tile([1, P], BF16)
        nc.vector.memset(onesc[:], 1.0)
        ones1_sb = constp.tile([1, C1], BF16)
        nc.sync.dma_start(ones1_sb[:], ones1_d[:])
        ones2_sb = constp.tile([1, C2], BF16)
        nc.sync.dma_start(ones2_sb[:], ones2_d[:])
        epsh = constp.tile([P, HEADS, 1], F32)
        nc.vector.memset(epsh[:], 1e-16)
        eps1 = constp.tile([P, 1], F32)
        nc.vector.memset(eps1[:], 1e-16)
        # persistent per-tile dst-side attention logits
        adst1_sb = constp.tile([P, NT, HEADS], BF16)
        adst2_sb = constp.tile([P, NT, 1], BF16)

        t1loc = drp.tile([NROWS, ROW1], BF16, name="t1loc")
        t2full = drp.tile([NROWS, ROW2], BF16, addr_space="Shared",
                          name="t2full")
        t2loc = drp.tile([NSLICE, ROW2], BF16, name="t2loc")

        # ---------------- Phase 0: replicated full-table build --------------
        UB = 8  # tiles per batched phase-0 DMA
        with tc.tile_pool(name="p0x", bufs=3) as px, \
             tc.tile_pool(name="p0t", bufs=3) as pt, \
             tc.tile_pool(name="p0c", bufs=1) as pc, \
             tc.tile_pool(name="p0p", bufs=2, space="PSUM") as pp:
            w1_sb = pc.tile([P, C1], BF16)
            nc.sync.dma_start(w1_sb[:], W1e_d[:])
            for tf8 in range(NTF // UB):
                xt8 = px.tile([P, UB, P], BF16)
                nc.sync.dma_start(
                    xt8[:],
                    xsT_d[:, tf8 * UB * P:(tf8 + 1) * UB * P]
                    .rearrange("f (u p) -> f u p", p=P))
                t1t8 = pt.tile([P, UB, ROW1], BF16)
                nc.vector.memset(t1t8[:], 0.0)
                for u in range(UB):
                    hps = pp.tile([P, C1], F32, space="PSUM")
                    nc.tensor.matmul(hps[:], lhsT=xt8[:, u, :], rhs=w1_sb[:],
                                     start=True, stop=False)
                    nc.tensor.matmul(hps[:], lhsT=onesc[:], rhs=ones1_sb[:],
                                     start=False, stop=True)
                    nc.scalar.activation(out=t1t8[:, u, 0:C1], in_=hps[:],
                                         func=ACT.Copy)
                nc.sync.dma_start(
                    t1loc[tf8 * UB * P:(tf8 + 1) * UB * P, :]
                    .rearrange("(u p) r -> p u r", p=P),
                    t1t8[:])

        # ---------------- Phase 1: layer-1 edges + fused table-2 build ------
        with tc.tile_pool(name="p1g", bufs=2) as pg, \
             tc.tile_pool(name="p1oh", bufs=2) as poh, \
             tc.tile_pool(name="p1ohT", bufs=2) as pohT, \
             tc.tile_pool(name="p1w", bufs=2) as pw, \
             tc.tile_pool(name="p1c", bufs=1) as pc, \
             tc.tile_pool(name="p1tp", bufs=2, space="PSUM") as ptp, \
             tc.tile_pool(name="p1ad", bufs=1, space="PSUM") as pad_, \
             tc.tile_pool(name="p1ac", bufs=2, space="PSUM") as pac, \
             tc.tile_pool(name="p1h2", bufs=1, space="PSUM") as ph2:
            b1_sb = pc.tile([P, D1I], F32)
            nc.sync.dma_start(b1_sb[:], b1_d[:])
            w2_sb = pc.tile([P, 3, C2], BF16)
            nc.sync.dma_start(w2_sb[:], W2e_d[:])

            # a_dst for local tiles, gathered from the table's adst columns
            for t in range(NT):
                nc.gpsimd.indirect_dma_start(
                    out=adst1_sb[:, t, :], out_offset=None,
                    in_=t1loc[:],
                    in_offset=IOA(ap=iloc_sb[:, t:t + 1], axis=0),
                    element_offset=264)

            for t in range(NT):
                cb = t * CPT
                G = pg.tile([P, CPT, ROW1], BF16)
                nc.gpsimd.dma_gather(
                    G[:], t1loc[:],
                    ixs_sb[:, t * CPT * 8:(t + 1) * CPT * 8],
                    CPT * P, CPT * P, ROW1, single_packet=False)
                oh = poh.tile([P, CPT, P], BF16)
                nc.sync.dma_start(
                    oh[:],
                    ohd_d[:, cb * P:(cb + CPT) * P]
                    .rearrange("p (c d) -> p c d", d=P))
                ohT = pohT.tile([P, CPT, P], BF16)
                for b in range(NTB):
                    c0, c1 = b * TB, min((b + 1) * TB, CPT)
                    tp = ptp.tile([P, TB, P], BF16, space="PSUM")
                    for c in range(c0, c1):
                        nc.tensor.transpose(tp[:, c - c0, :], oh[:, c, :],
                                            identb[:])
                    nc.vector.tensor_copy(out=ohT[:, c0:c1, :],
                                          in_=tp[:, 0:c1 - c0, :])
                adps = pad_.tile([P, CPT, HEADS], F32, space="PSUM")
                for c in range(CPT):
                    nc.tensor.matmul(adps[:, c, :], lhsT=ohT[:, c, :],
                                     rhs=adst1_sb[:, t, :],
                                     start=True, stop=True)
                # z = asrc + adps + w*q ; p = max(exp(z), exp(0.2 z))
                asr = pw.tile([P, CPT, HEADS], F32)
                nc.scalar.activation(out=asr[:], in_=G[:, :, 260:264],
                                     func=ACT.Copy)
                nc.vector.tensor_tensor(out=asr[:], in0=asr[:], in1=adps[:],
                                        op=A.add)
                nc.vector.tensor_tensor(out=asr[:], in0=asr[:],
                                        in1=wvq1_sb[:, cb:cb + CPT, :],
                                        op=A.add)
                e2 = pw.tile([P, CPT, HEADS], F32)
                nc.scalar.activation(out=e2[:], in_=asr[:], func=ACT.Exp,
                                     scale=0.2)
                nc.scalar.activation(out=asr[:], in_=asr[:], func=ACT.Exp)
                nc.vector.tensor_tensor(out=asr[:], in0=asr[:], in1=e2[:],
                                        op=A.max)
                nc.scalar.activation(out=G[:, :, 268:272], in_=asr[:],
                                     func=ACT.Copy)
                gm = G[:, :, 0:D1I].rearrange("p c (h k) -> p c h k", h=HEADS)
                nc.vector.tensor_tensor(
                    out=gm, in0=gm,
                    in1=G[:, :, 268:272].unsqueeze(3)
                        .to_broadcast([P, CPT, HEADS, 65]),
                    op=A.mult)
                acc = pac.tile([P, D1I], F32, space="PSUM")
                for c in range(CPT):
                    nc.tensor.matmul(acc[:], lhsT=oh[:, c, :],
                                     rhs=G[:, c, 0:D1I],
                                     start=(c == 0), stop=(c == CPT - 1))
                # epilogue: out1 = relu(acc/denom + b1)  (interleaved layout)
                accv = acc[:].rearrange("p (h k) -> p h k", h=HEADS)
                dn = pw.tile([P, HEADS, 1], F32)
                nc.vector.tensor_tensor(out=dn[:], in0=accv[:, :, 64:65],
                                        in1=epsh[:], op=A.add)
                rc = pw.tile([P, HEADS, 1], F32)
                nc.vector.reciprocal(rc[:], dn[:])
                ob = pw.tile([P, D1I], F32)
                obv = ob[:].rearrange("p (h k) -> p h k", h=HEADS)
                nc.vector.tensor_tensor(
                    out=obv, in0=accv,
                    in1=rc[:].to_broadcast([P, HEADS, 65]),
                    op=A.mult)
                nc.vector.tensor_tensor(out=ob[:], in0=ob[:], in1=b1_sb[:],
                                        op=A.add)
                o1t = pw.tile([P, 3 * P], BF16)
                nc.scalar.activation(out=o1t[:, 0:D1I], in_=ob[:],
                                     func=ACT.Relu)
                nc.vector.memset(o1t[:, D1I:3 * P], 0.0)
                # h2 = o1t @ W2e  (3 K-chunks of 128 + ones row)
                h2ps = ph2.tile([P, C2], F32, space="PSUM")
                for k in range(3):
                    tpo = ptp.tile([P, P], BF16, space="PSUM")
                    nc.tensor.transpose(tpo[:],
                                        o1t[:, k * P:(k + 1) * P], identb[:])
                    o1T = pw.tile([P, P], BF16)
                    nc.vector.tensor_copy(out=o1T[:], in_=tpo[:])
                    nc.tensor.matmul(h2ps[:], lhsT=o1T[:],
                                     rhs=w2_sb[:, k, :],
                                     start=(k == 0), stop=False)
                nc.tensor.matmul(h2ps[:], lhsT=onesc[:], rhs=ones2_sb[:],
                                 start=False, stop=True)
                t2t = pw.tile([P, ROW2], BF16)
                nc.scalar.activation(out=t2t[:, 0:C2], in_=h2ps[:],
                                     func=ACT.Copy)
                nc.vector.memset(t2t[:, C2:ROW2], 0.0)
                nc.scalar.activation(out=adst2_sb[:, t, :],
                                     in_=h2ps[:, 66:67], func=ACT.Copy)
                nc.sync.dma_start(t2loc[t * P:(t + 1) * P, :], t2t[:])
            nc.gpsimd.collective_compute(
                "AllGather", A.bypass, replica_groups=rg,
                ins=[t2loc[:]], outs=[t2full[:]])

        # ---------------- Phase 2: layer-2 edges + pooling ------------------
        with tc.tile_pool(name="p2g", bufs=2) as pg, \
             tc.tile_pool(name="p2oh", bufs=2) as poh, \
             tc.tile_pool(name="p2ohT", bufs=2) as pohT, \
             tc.tile_pool(name="p2w", bufs=2) as pw, \
             tc.tile_pool(name="p2c", bufs=1) as pc, \
             tc.tile_pool(name="p2ad", bufs=2, space="PSUM") as pad2, \
             tc.tile_pool(name="p2ac", bufs=2, space="PSUM") as pac, \
             tc.tile_pool(name="p2tl", bufs=1, space="PSUM") as ptl, \
             tc.tile_pool(name="p2pl", bufs=1, space="PSUM") as ppl:
            b2_sb = pc.tile([P, HID], F32)
            nc.sync.dma_start(b2_sb[:], b2_d[:])
            pool_ps = ppl.tile([GPC, HID + 1], F32, space="PSUM")
            for t in range(NT):
                cb = t * CPT
                G2 = pg.tile([P, CPT, ROW2], BF16)
                nc.gpsimd.dma_gather(
                    G2[:], t2full[:],
                    ixs_sb[:, t * CPT * 8:(t + 1) * CPT * 8],
                    CPT * P, CPT * P, ROW2, single_packet=False)
                oh = poh.tile([P, CPT, P], BF16)
                nc.sync.dma_start(
                    oh[:],
                    ohd_d[:, cb * P:(cb + CPT) * P]
                    .rearrange("p (c d) -> p c d", d=P))
                ohT = pohT.tile([P, CPT, P], BF16)
                nc.sync.dma_start(
                    ohT[:],
                    ohTd_d[:, cb * P:(cb + CPT) * P]
                    .rearrange("p (c d) -> p c d", d=P))
                adps2 = pad2.tile([P, CPT, 1], F32, space="PSUM")
                for c in range(CPT):
                    nc.tensor.matmul(adps2[:, c, :], lhsT=ohT[:, c, :],
                                     rhs=adst2_sb[:, t, :],
                                     start=True, stop=True)
                asr = pw.tile([P, CPT, 1], F32)
                nc.scalar.activation(out=asr[:], in_=G2[:, :, 65:66],
                                     func=ACT.Copy)
                nc.vector.tensor_tensor(out=asr[:], in0=asr[:], in1=adps2[:],
                                        op=A.add)
                nc.vector.tensor_tensor(out=asr[:], in0=asr[:],
                                        in1=wvq2_sb[:, cb:cb + CPT]
                                        .unsqueeze(2),
                                        op=A.add)
                e2 = pw.tile([P, CPT, 1], F32)
                nc.scalar.activation(out=e2[:], in_=asr[:], func=ACT.Exp,
                                     scale=0.2)
                nc.scalar.activation(out=asr[:], in_=asr[:], func=ACT.Exp)
                nc.vector.tensor_tensor(out=asr[:], in0=asr[:], in1=e2[:],
                                        op=A.max)
                nc.scalar.activation(out=G2[:, :, 67:68], in_=asr[:],
                                     func=ACT.Copy)
                nc.vector.tensor_tensor(
                    out=G2[:, :, 0:65], in0=G2[:, :, 0:65],
                    in1=G2[:, :, 67:68].to_broadcast([P, CPT, 65]),
                    op=A.mult)
                acc = pac.tile([P, HID + 1], F32, space="PSUM")
                for c in range(CPT):
                    nc.tensor.matmul(acc[:], lhsT=oh[:, c, :],
                                     rhs=G2[:, c, 0:HID + 1],
                                     start=(c == 0), stop=(c == CPT - 1))
                dn = pw.tile([P, 1], F32)
                nc.vector.tensor_tensor(out=dn[:], in0=acc[:, HID:HID + 1],
                                        in1=eps1[:], op=A.add)
                rc = pw.tile([P, 1], F32)
                nc.vector.reciprocal(rc[:], dn[:])
                o2 = pw.tile([P, HID], F32)
                nc.vector.tensor_tensor(
                    out=o2[:], in0=acc[:, 0:HID],
                    in1=rc[:].to_broadcast([P, HID]), op=A.mult)
                nc.vector.tensor_tensor(out=o2[:], in0=o2[:], in1=b2_sb[:],
                                        op=A.add)
                o2t = pw.tile([P, HID + 1], BF16)
                nc.scalar.activation(out=o2t[:, 0:HID], in_=o2[:],
                                     func=ACT.Relu)
                nc.vector.memset(o2t[:, HID:HID + 1], 1.0)
                nc.tensor.matmul(pool_ps[:], lhsT=ohg_sb[:, t, :],
                                 rhs=o2t[:],
                                 start=(t == 0), stop=(t == NT - 1),
                                 skip_group_check=True)

            # ------------- Phase 3: pooled mean + FC ------------------------
            fcw_sb = pc.tile([HID, OUT], F32)
            nc.sync.dma_start(fcw_sb[:], fcw_d[:])
            fcb_sb = pc.tile([P, OUT], F32)
            nc.sync.dma_start(fcb_sb[:], fcb_d[:])
            one_g = pc.tile([GPC, 1], F32)
            nc.vector.memset(one_g[:], 1.0)
            cnt = pc.tile([GPC, 1], F32)
            nc.vector.tensor_tensor(out=cnt[:], in0=pool_ps[:, HID:HID + 1],
                                    in1=one_g[:], op=A.max)
            rcc = pc.tile([GPC, 1], F32)
            nc.vector.reciprocal(rcc[:], cnt[:])
            pooled = pc.tile([GPC, HID], F32)
            nc.vector.tensor_tensor(out=pooled[:], in0=pool_ps[:, 0:HID],
                                    in1=rcc[:].to_broadcast([GPC, HID]),
                                    op=A.mult)
            pT_ps = ptl.tile([HID, GPC], F32, space="PSUM")
            nc.tensor.transpose(pT_ps[:], pooled[:], identf[:GPC, :GPC])
            pT = pc.tile([HID, GPC], F32)
            nc.vector.tensor_copy(out=pT[:], in_=pT_ps[:])
            fc_ps = ptl.tile([GPC, OUT], F32, space="PSUM")
            nc.tensor.matmul(fc_ps[:], lhsT=pT[:], rhs=fcw_sb[:],
                             start=True, stop=True)
            res = pc.tile([GPC, OUT], F32)
            nc.vector.tensor_tensor(out=res[:], in0=fc_ps[:],
                                    in1=fcb_sb[:GPC, :], op=A.add)
            nc.sync.dma_start(out_d[:], res[:])

    nc.compile()
    return nc


# ---------------------------------------------------------------------------
# Entry point.
# ---------------------------------------------------------------------------
def run(inputs, cfg, **run_kwargs):
    in_maps, meta = prepare(inputs, cfg)
    nc = build(meta)
    res = run_bass_kernel_spmd(nc, in_maps, core_ids=list(range(NCORES)),
                               **run_kwargs)
    out = np.concatenate([res.results[c]["out"] for c in range(NCORES)],
                         axis=0)
    return np.asarray(out, np.float32), res


def kernel(**inputs) -> np.ndarray:
    out, _ = run(inputs, FULL_CFG)
    return out
